# revision 5
# baseline (speedup 1.0000x reference)
"""AGCRN (adaptive graph conv GRU) Trainium2 kernel, v2 node-major.

Model (B=64, L=24, N=512, F=2, H=128, ED=16, HOR=12):
  A = softmax(relu((E@W1)(E@W2)^T))                       [N,N]
  scan over L:  inp=[x_t, h];  g=A@inp;  z=sig(g@Wz+bz); r=sig(g@Wr+br)
                gh=A@[x_t, r*h]; ht=tanh(gh@Wh+bh); h=(1-z)h+z*ht
  out = (h@Whead + bhead) transposed to [B, HOR, N]

Distribution: pure data-parallel over batch B across 8 NeuronCores
(8 batches/core), weights + A replicated, no collectives.

v2 design (vs baseline ^T layout):
  - State kept NODE-major only: hbf [128(node%128), b, c(node//128), H].
    Gates computed node-major (lhsT = G^T column-slices), so the GRU
    update and r*h need NO per-step PE transposes.
  - Gate bias + x-contribution in ONE K=52 matmul: axt partitions
    64*Gr + 13k + (2j+f) hold (A@x_t)^T rows for t=6k+j; partition
    13k+12 holds the exact softmax row-sums (=1.0) -> free bias row.
    The weight side is a per-t zero-padded [52, *] variant; two of
    these matmuls pack concurrently via tile_position rows {0, 64}.
  - z|r fused in one [128, 2, 4, 256] psum pair-tile -> single sigmoid
    over FD=2048; tanh over a pair-tile FD=1024.
  - Optional fp8 e4m3 + DoubleRow convs (A scaled x64, 1/64 folded
    into Wzh/Wrh/Whh host-side).
  - PSUM evacuation split by columns between ACT and DVE; r*h and the
    fp8 state cast on GpSimd.
"""
import numpy as np
from contextlib import ExitStack as _ExitStack

import concourse.bass as bass
import concourse.mybir as mybir
import concourse.tile as tile
from concourse import bacc
from concourse.bass_utils import run_bass_kernel_spmd
from concourse.masks import make_identity

F32 = mybir.dt.float32
F32R = mybir.dt.float32r
BF16 = mybir.dt.bfloat16
FP8 = mybir.dt.float8e4
AF = mybir.ActivationFunctionType
ALU = mybir.AluOpType

B, L, N, F, H, ED, HOR = 64, 24, 512, 2, 128, 16, 12
NCORES = 8
NB = B // NCORES          # batches per core
NC4 = N // 128            # node chunks
NK, NJ = 4, 6             # t = 6*k + j
KX = 52                   # x-side contraction: 4k * (12 jf + 1 ones)

USE_FP8 = False           # fp8+DoubleRow convolutions (A scaled x64)
ASCALE = 64.0
ACT_COPY_COLS = 128       # G/Gh psum->sbuf: cols [0:ACT_COPY_COLS] on ACT


def build_nc():
    nc = bacc.Bacc("TRN2", target_bir_lowering=False, debug=False,
                   num_devices=NCORES)

    xn_ext = nc.declare_dram_parameter("Xn", [NC4, 128, NB, NK, 13], F32,
                                       isOutput=False)
    et_ext = nc.declare_dram_parameter("ET", [ED, N], F32, isOutput=False)
    w1_ext = nc.declare_dram_parameter("W1", [ED, ED], F32, isOutput=False)
    w2_ext = nc.declare_dram_parameter("W2", [ED, ED], F32, isOutput=False)
    wzr_ext = nc.declare_dram_parameter("WZR", [H, 2 * H], F32, isOutput=False)
    whh_ext = nc.declare_dram_parameter("WHH", [H, H], F32, isOutput=False)
    wxzr_ext = nc.declare_dram_parameter("WXZR", [128, L, 2 * H], F32,
                                         isOutput=False)
    wxh_ext = nc.declare_dram_parameter("WXH", [128, L, H], F32,
                                        isOutput=False)
    whd_ext = nc.declare_dram_parameter("Whead", [H, HOR], F32, isOutput=False)
    bhd_ext = nc.declare_dram_parameter("bhead", [HOR], F32, isOutput=False)
    out_ext = nc.declare_dram_parameter("out", [NB, HOR, N], F32, isOutput=True)

    with tile.TileContext(nc) as tc:
        with tc.tile_pool(name="const", bufs=1) as cpool:
            ident = cpool.tile([128, 128], F32, tag="ident")
            make_identity(nc, ident[:])
            ident_b = cpool.tile([128, 128], BF16, tag="identb")
            nc.vector.tensor_copy(ident_b[:], ident[:])

            wzr_sb = cpool.tile([H, 2 * H], BF16, tag="wzr")
            whh_sb = cpool.tile([H, H], BF16, tag="whh")
            wxzr_sb = cpool.tile([128, L, 2 * H], BF16, tag="wxzr")
            wxh_sb = cpool.tile([128, L, H], BF16, tag="wxh")
            whd_sb = cpool.tile([H, HOR], BF16, tag="whd")
            bhd_sb = cpool.tile([HOR, 1], F32, tag="bhd")
            nc.gpsimd.dma_start(wzr_sb[:], wzr_ext[:])
            nc.gpsimd.dma_start(whh_sb[:], whh_ext[:])
            nc.gpsimd.dma_start(wxzr_sb[:], wxzr_ext[:])
            nc.gpsimd.dma_start(wxh_sb[:], wxh_ext[:])
            nc.gpsimd.dma_start(whd_sb[:], whd_ext[:])
            nc.sync.dma_start(bhd_sb[:], bhd_ext[:].unsqueeze(-1))

            # conv moving operand: A^T chunks (bf16 + optional fp8 x64)
            AT = cpool.tile([128, NC4, N], BF16, tag="AT")
            if USE_FP8:
                at8 = cpool.tile([128, 2, 2, N], FP8, tag="at8")
            # x side: axt[64Gr + 13k + (2j+f), b, n] = (A@x_t)^T[f, n]
            # for t=6k+j; partition 13k+12 = softmax row sums (=1.0)
            axt = cpool.tile([128, NB, N], BF16, tag="axt")

            # ---- adjacency precompute ----
            with tc.tile_pool(name="pre", bufs=1) as pre:
                with tc.tile_pool(name="ppreA", bufs=2,
                                  space="PSUM") as ppre:
                    et_sb = pre.tile([ED, N], F32R, tag="et")
                    w1_sb = pre.tile([ED, ED], F32R, tag="w1")
                    w2_sb = pre.tile([ED, ED], F32R, tag="w2")
                    nc.sync.dma_start(et_sb[:], et_ext[:].bitcast(F32R))
                    nc.sync.dma_start(w1_sb[:], w1_ext[:].bitcast(F32R))
                    nc.sync.dma_start(w2_sb[:], w2_ext[:].bitcast(F32R))

                    m1t = pre.tile([ED, N], F32R, tag="m1t")
                    m2t = pre.tile([ED, N], F32R, tag="m2t")
                    for wsb, mt in ((w1_sb, m1t), (w2_sb, m2t)):
                        ps = ppre.tile([ED, N], F32, tag="mps")
                        nc.tensor.matmul(ps[:], wsb[:], et_sb[:], start=True,
                                         stop=True)
                        nc.scalar.copy(mt[:], ps[:])

                    A_sb = [pre.tile([128, N], F32, tag=f"A{i}", name=f"A{i}")
                            for i in range(NC4)]
                    for i in range(NC4):
                        ps = ppre.tile([128, N], F32, tag="sps")
                        nc.tensor.matmul(ps[:], m1t[:, i * 128:(i + 1) * 128],
                                         m2t[:], start=True, stop=True)
                        s_sb = pre.tile([128, N], F32, tag="s")
                        nc.scalar.activation(s_sb[:], ps[:], AF.Relu)
                        mx = pre.tile([128, 1], F32, tag="mx")
                        nc.vector.tensor_reduce(mx[:], s_sb[:],
                                                axis=mybir.AxisListType.X,
                                                op=ALU.max)
                        nmx = pre.tile([128, 1], F32, tag="nmx")
                        nc.vector.tensor_scalar_mul(nmx[:], mx[:], -1.0)
                        sm = pre.tile([128, 1], F32, tag="sm")
                        nc.scalar.activation(A_sb[i][:], s_sb[:], AF.Exp,
                                             bias=nmx[:], accum_out=sm[:])
                        rs = pre.tile([128, 1], F32, tag="rs")
                        nc.vector.reciprocal(rs[:], sm[:])
                        nc.vector.tensor_scalar_mul(A_sb[i][:], A_sb[i][:],
                                                    rs[:])

                    # AT chunks via 16 PE transposes
                    for c in range(NC4):
                        for i in range(NC4):
                            tp = ppre.tile([128, 128], F32, tag="tp")
                            nc.tensor.transpose(
                                tp[:], A_sb[i][:, c * 128:(c + 1) * 128],
                                ident[:])
                            nc.vector.tensor_copy(
                                AT[:, c, i * 128:(i + 1) * 128], tp[:])
                            if USE_FP8:
                                nc.scalar.mul(at8[:, c // 2, c % 2,
                                                  i * 128:(i + 1) * 128],
                                              tp[:], ASCALE)

                # ---- AXT precompute: (A@x_t)^T rows ----
                with tc.tile_pool(name="ppreB", bufs=2,
                                  space="PSUM") as ppreB:
                    xn_sb = [pre.tile([128, NB, NK, 13], BF16, tag=f"xn{c}",
                                      name=f"xn{c}") for c in range(NC4)]
                    for c in range(NC4):
                        nc.gpsimd.dma_start(xn_sb[c][:], xn_ext[c])
                    for b in range(NB):
                        axps = ppreB.tile([KX, N], F32, tag="axps",
                                          name=f"axps{b}")
                        for c in range(NC4):
                            nc.tensor.matmul(axps[:], xn_sb[c][:, b, :, :],
                                             AT[:, c, :], start=(c == 0),
                                             stop=(c == NC4 - 1))
                        nc.vector.tensor_copy(axt[0:KX, b, :], axps[:])

            # ---- scan over L ----
            _st = _ExitStack()
            spool = _st.enter_context(tc.tile_pool(name="state", bufs=2))
            wpool = _st.enter_context(tc.tile_pool(name="work", bufs=2))
            pconv = _st.enter_context(
                tc.tile_pool(name="pconv", bufs=2, space="PSUM"))
            pzr = _st.enter_context(
                tc.tile_pool(name="pzr", bufs=1, space="PSUM"))
            pht = _st.enter_context(
                tc.tile_pool(name="pht", bufs=1, space="PSUM"))

            hbf_prev = None
            h8_prev = None
            CONV8 = USE_FP8
            DR = mybir.MatmulPerfMode.DoubleRow
            for t in range(L):
                hbf_new = spool.tile([128, NB, NC4, H], BF16, tag="hbf",
                                     name=f"hbf{t}")
                if CONV8:
                    h8_new = spool.tile([128, NB, NC4, H], FP8, tag="h8",
                                        name=f"h8{t}")
                for p in range(NB // 2):
                    b0 = 2 * p
                    zrps = pzr.tile([128, 2, NC4, 2 * H], F32, tag="zr",
                                    name=f"zrps{t}_{p}")
                    htps = pht.tile([128, 2, NC4, H], F32, tag="ht",
                                    name=f"htps{t}_{p}")
                    gsb = {}
                    for bi in range(2):
                        b = b0 + bi
                        if t > 0:
                            # conv1: G^T = (A @ h)^T accumulated in psum
                            gps = pconv.tile([H, N], F32, tag="cv",
                                             name=f"gps{t}_{b}")
                            if CONV8:
                                for p2 in range(2):
                                    nc.tensor.matmul(
                                        gps[:],
                                        h8_prev[:, b, 2 * p2:2 * p2 + 2, :],
                                        at8[:, p2, :, :],
                                        start=(p2 == 0), stop=(p2 == 1),
                                        perf_mode=DR)
                            else:
                                for c in range(NC4):
                                    nc.tensor.matmul(
                                        gps[:], hbf_prev[:, b, c, :],
                                        AT[:, c, :],
                                        start=(c == 0), stop=(c == NC4 - 1))
                            g_sb = wpool.tile([H, N], BF16, tag="gsb",
                                              name=f"gsb{t}_{b}")
                            nc.scalar.copy(g_sb[:, 0:ACT_COPY_COLS],
                                           gps[:, 0:ACT_COPY_COLS])
                            nc.vector.tensor_copy(g_sb[:, ACT_COPY_COLS:],
                                                  gps[:, ACT_COPY_COLS:])
                            gsb[bi] = g_sb
                        # z|r gates: per bank, x(+bias) then main
                        for ca, cb in ((0, 2), (1, 3)):
                            for c in (ca, cb):
                                nc.tensor.matmul(
                                    zrps[:, bi, c, :],
                                    axt[0:KX, b,
                                        c * 128:(c + 1) * 128],
                                    wxzr_sb[0:KX, t, :],
                                    start=True, stop=(t == 0))
                            if t > 0:
                                for c in (ca, cb):
                                    nc.tensor.matmul(
                                        zrps[:, bi, c, :],
                                        gsb[bi][:, c * 128:(c + 1) * 128],
                                        wzr_sb[:], start=False, stop=True)
                    # sigmoid over the whole pair tile (z and r)
                    zrsb = wpool.tile([128, 2, NC4, 2 * H], BF16, tag="zrsb",
                                      name=f"zrsb{t}_{p}")
                    nc.scalar.activation(zrsb[:], zrps[:], AF.Sigmoid)
                    # r*h -> conv2 lhsT (gpsimd, off the DVE/ACT hot path)
                    if t > 0:
                        rh = spool.tile([128, 2, NC4, H],
                                        FP8 if CONV8 else BF16, tag="rh",
                                        name=f"rh{t}_{p}")
                        nc.gpsimd.tensor_tensor(
                            rh[:], zrsb[:, :, :, H:2 * H],
                            hbf_prev[:, b0:b0 + 2, :, :], ALU.mult)
                    for bi in range(2):
                        b = b0 + bi
                        if t > 0:
                            ghps = pconv.tile([H, N], F32, tag="cv",
                                              name=f"ghps{t}_{b}")
                            if CONV8:
                                for p2 in range(2):
                                    nc.tensor.matmul(
                                        ghps[:],
                                        rh[:, bi, 2 * p2:2 * p2 + 2, :],
                                        at8[:, p2, :, :],
                                        start=(p2 == 0), stop=(p2 == 1),
                                        perf_mode=DR)
                            else:
                                for c in range(NC4):
                                    nc.tensor.matmul(
                                        ghps[:], rh[:, bi, c, :], AT[:, c, :],
                                        start=(c == 0), stop=(c == NC4 - 1))
                            gh_sb = wpool.tile([H, N], BF16, tag="ghsb",
                                               name=f"ghsb{t}_{b}")
                            nc.scalar.copy(gh_sb[:, 0:ACT_COPY_COLS],
                                           ghps[:, 0:ACT_COPY_COLS])
                            nc.vector.tensor_copy(gh_sb[:, ACT_COPY_COLS:],
                                                  ghps[:, ACT_COPY_COLS:])
                        # candidate gate: x(+bias) then main, bank-serial
                        for c in range(NC4):
                            nc.tensor.matmul(
                                htps[:, bi, c, :],
                                axt[0:KX, b,
                                    c * 128:(c + 1) * 128],
                                wxh_sb[0:KX, t, :],
                                start=True, stop=(t == 0))
                            if t > 0:
                                nc.tensor.matmul(
                                    htps[:, bi, c, :],
                                    gh_sb[:, c * 128:(c + 1) * 128],
                                    whh_sb[:], start=False, stop=True)
                    htsb = wpool.tile([128, 2, NC4, H], BF16, tag="htsb",
                                      name=f"htsb{t}_{p}")
                    nc.scalar.activation(htsb[:], htps[:], AF.Tanh)
                    # GRU update: hnew = z*ht - (z-1)*h
                    z_ap = zrsb[:, :, :, 0:H]
                    hn_ap = hbf_new[:, b0:b0 + 2, :, :]
                    if t == 0:
                        nc.vector.tensor_mul(hn_ap, z_ap, htsb[:])
                    else:
                        u = wpool.tile([128, 2, NC4, H], BF16, tag="u",
                                       name=f"u{t}_{p}")
                        nc.vector.scalar_tensor_tensor(
                            u[:], z_ap, 1.0, hbf_prev[:, b0:b0 + 2, :, :],
                            ALU.subtract, ALU.mult)
                        v = wpool.tile([128, 2, NC4, H], BF16, tag="v",
                                       name=f"v{t}_{p}")
                        nc.vector.tensor_mul(v[:], z_ap, htsb[:])
                        nc.vector.tensor_sub(hn_ap, v[:], u[:])
                    if CONV8:
                        nc.gpsimd.tensor_copy(h8_new[:, b0:b0 + 2, :, :],
                                              hn_ap)
                hbf_prev = hbf_new
                if CONV8:
                    h8_prev = h8_new

            _st.close()

            # ---- head: out = (h @ Whead + bhead)^T ----
            with tc.tile_pool(name="hd", bufs=2) as hp, \
                 tc.tile_pool(name="phd", bufs=2, space="PSUM") as ph:
                for b in range(NB):
                    tp = ph.tile([128, N], BF16, tag="tp", name=f"tp{b}")
                    for c in range(NC4):
                        nc.tensor.transpose(tp[:, c * 128:(c + 1) * 128],
                                            hbf_prev[:, b, c, :], ident_b[:])
                    hT = hp.tile([128, N], BF16, tag="hT", name=f"hT{b}")
                    nc.vector.tensor_copy(hT[:], tp[:])
                    hd = ph.tile([HOR, N], F32, tag="hd", name=f"hd{b}")
                    nc.tensor.matmul(hd[:], whd_sb[:], hT[:], start=True,
                                     stop=True)
                    o_sb = hp.tile([HOR, N], F32, tag="o", name=f"o{b}")
                    nc.scalar.activation(o_sb[:], hd[:], AF.Identity,
                                         bias=bhd_sb[:])
                    nc.sync.dma_start(out_ext[b], o_sb[:])

    nc.compile()
    return nc


_NC_CACHE = None


def _get_nc():
    global _NC_CACHE
    if _NC_CACHE is None:
        _NC_CACHE = build_nc()
    return _NC_CACHE


def make_in_maps(X, E, W1, W2, Wz, bz, Wr, br, Wh, bh, Whead, bhead):
    f32 = np.float32
    Wz, Wr, Wh = np.asarray(Wz, f32), np.asarray(Wr, f32), np.asarray(Wh, f32)
    bz, br, bh = np.asarray(bz, f32), np.asarray(br, f32), np.asarray(bh, f32)
    scale = 1.0 / ASCALE if USE_FP8 else 1.0
    wzr = np.concatenate([Wz[F:], Wr[F:]], axis=1) * scale       # [H, 2H]
    whh = Wh[F:] * scale                                          # [H, H]
    # per-t zero-padded x-side stationaries over the K=52 axt rows
    wxzr = np.zeros((128, L, 2 * H), f32)
    wxh = np.zeros((128, L, H), f32)
    xzr = np.concatenate([Wz[0:F], Wr[0:F]], axis=1)              # [2, 2H]
    bzr = np.concatenate([bz, br])                                # [2H]
    for gr in range(2):
        for t in range(L):
            k, j = t // NJ, t % NJ
            r0 = 64 * gr + 13 * k
            wxzr[r0 + 2 * j:r0 + 2 * j + 2, t, :] = xzr
            wxzr[r0 + 12, t, :] = bzr
            wxh[r0 + 2 * j:r0 + 2 * j + 2, t, :] = Wh[0:F]
            wxh[r0 + 12, t, :] = bh
    ET = np.ascontiguousarray(np.asarray(E, f32).T)
    common = {
        "ET": ET, "W1": np.asarray(W1, f32), "W2": np.asarray(W2, f32),
        "WZR": np.ascontiguousarray(wzr), "WHH": np.ascontiguousarray(whh),
        "WXZR": wxzr, "WXH": wxh,
        "Whead": np.asarray(Whead, f32), "bhead": np.asarray(bhead, f32),
    }
    X = np.asarray(X, f32)
    in_maps = []
    for i in range(NCORES):
        xs = X[i * NB:(i + 1) * NB]                     # [NB, L, N, F]
        # xn[c, ki, b, k, 2j+f] = xs[b, 6k+j, 128c+ki, f]; col 12 = 1.0
        xr = xs.transpose(2, 0, 1, 3).reshape(NC4, 128, NB, NK, NJ * F)
        xn = np.zeros((NC4, 128, NB, NK, 13), f32)
        xn[..., :NJ * F] = xr
        xn[..., 12] = 1.0
        in_maps.append({"Xn": xn, **common})
    return in_maps


def run_on_hw(inputs, **kwargs):
    nc = _get_nc()
    in_maps = make_in_maps(**inputs)
    last_err = None
    for _attempt in range(3):
        try:
            res = run_bass_kernel_spmd(nc, in_maps,
                                       core_ids=list(range(NCORES)), **kwargs)
            break
        except Exception as e:  # transient NRT_EXEC_UNIT_UNRECOVERABLE
            last_err = e
            if "UNRECOVERABLE" not in str(e) and "UNAVAILABLE" not in str(e):
                raise
    else:
        raise last_err
    out = np.concatenate([res.results[i]["out"] for i in range(NCORES)], axis=0)
    return out, res


def kernel(**inputs) -> np.ndarray:
    out, _ = run_on_hw(inputs)
    return out


# revision 6
# speedup vs baseline: 1.7479x; 1.7479x over previous
"""AGCRN (adaptive graph conv GRU) Trainium2 kernel, v2 node-major.

Model (B=64, L=24, N=512, F=2, H=128, ED=16, HOR=12):
  A = softmax(relu((E@W1)(E@W2)^T))                       [N,N]
  scan over L:  inp=[x_t, h];  g=A@inp;  z=sig(g@Wz+bz); r=sig(g@Wr+br)
                gh=A@[x_t, r*h]; ht=tanh(gh@Wh+bh); h=(1-z)h+z*ht
  out = (h@Whead + bhead) transposed to [B, HOR, N]

Distribution: pure data-parallel over batch B across 8 NeuronCores
(8 batches/core), weights + A replicated, no collectives.

v2 design (vs baseline ^T layout):
  - State kept NODE-major only: hbf [128(node%128), b, c(node//128), H].
    Gates computed node-major (lhsT = G^T column-slices), so the GRU
    update and r*h need NO per-step PE transposes.
  - Gate bias + x-contribution in ONE K=52 matmul: axt partitions
    64*Gr + 13k + (2j+f) hold (A@x_t)^T rows for t=6k+j; partition
    13k+12 holds the exact softmax row-sums (=1.0) -> free bias row.
    The weight side is a per-t zero-padded [52, *] variant; two of
    these matmuls pack concurrently via tile_position rows {0, 64}.
  - z|r fused in one [128, 2, 4, 256] psum pair-tile -> single sigmoid
    over FD=2048; tanh over a pair-tile FD=1024.
  - Optional fp8 e4m3 + DoubleRow convs (A scaled x64, 1/64 folded
    into Wzh/Wrh/Whh host-side).
  - PSUM evacuation split by columns between ACT and DVE; r*h and the
    fp8 state cast on GpSimd.
"""
import numpy as np
from contextlib import ExitStack as _ExitStack

import concourse.bass as bass
import concourse.mybir as mybir
import concourse.tile as tile
from concourse import bacc
from concourse.bass_utils import run_bass_kernel_spmd
from concourse.masks import make_identity

F32 = mybir.dt.float32
F32R = mybir.dt.float32r
BF16 = mybir.dt.bfloat16
FP8 = mybir.dt.float8e4
AF = mybir.ActivationFunctionType
ALU = mybir.AluOpType

B, L, N, F, H, ED, HOR = 64, 24, 512, 2, 128, 16, 12
NCORES = 8
NB = B // NCORES          # batches per core
NC4 = N // 128            # node chunks
NK, NJ = 4, 6             # t = 6*k + j
KX = 52                   # x-side contraction: 4k * (12 jf + 1 ones)

USE_FP8 = False           # fp8+DoubleRow convolutions (A scaled x64)
ASCALE = 64.0
ACT_COPY_COLS = 128       # G/Gh psum->sbuf: cols [0:ACT_COPY_COLS] on ACT


def build_nc():
    nc = bacc.Bacc("TRN2", target_bir_lowering=False, debug=False,
                   num_devices=NCORES)

    xn_ext = nc.declare_dram_parameter("Xn", [NC4, 128, NB, NK, 13], F32,
                                       isOutput=False)
    et_ext = nc.declare_dram_parameter("ET", [ED, N], F32, isOutput=False)
    w1_ext = nc.declare_dram_parameter("W1", [ED, ED], F32, isOutput=False)
    w2_ext = nc.declare_dram_parameter("W2", [ED, ED], F32, isOutput=False)
    wzr_ext = nc.declare_dram_parameter("WZR", [H, 2 * H], F32, isOutput=False)
    whh_ext = nc.declare_dram_parameter("WHH", [H, H], F32, isOutput=False)
    wxzr_ext = nc.declare_dram_parameter("WXZR", [128, L, 2 * H], F32,
                                         isOutput=False)
    wxh_ext = nc.declare_dram_parameter("WXH", [128, L, H], F32,
                                        isOutput=False)
    whd_ext = nc.declare_dram_parameter("Whead", [H, HOR], F32, isOutput=False)
    bhd_ext = nc.declare_dram_parameter("bhead", [HOR], F32, isOutput=False)
    out_ext = nc.declare_dram_parameter("out", [NB, HOR, N], F32, isOutput=True)

    with tile.TileContext(nc) as tc:
        with tc.tile_pool(name="const", bufs=1) as cpool:
            ident = cpool.tile([128, 128], F32, tag="ident")
            make_identity(nc, ident[:])
            ident_b = cpool.tile([128, 128], BF16, tag="identb")
            nc.vector.tensor_copy(ident_b[:], ident[:])

            wzr_sb = cpool.tile([H, 2 * H], BF16, tag="wzr")
            whh_sb = cpool.tile([H, H], BF16, tag="whh")
            wxzr_sb = cpool.tile([128, L, 2 * H], BF16, tag="wxzr")
            wxh_sb = cpool.tile([128, L, H], BF16, tag="wxh")
            whd_sb = cpool.tile([H, HOR], BF16, tag="whd")
            bhd_sb = cpool.tile([HOR, 1], F32, tag="bhd")
            nc.gpsimd.dma_start(wzr_sb[:], wzr_ext[:])
            nc.gpsimd.dma_start(whh_sb[:], whh_ext[:])
            nc.gpsimd.dma_start(wxzr_sb[:], wxzr_ext[:])
            nc.gpsimd.dma_start(wxh_sb[:], wxh_ext[:])
            nc.gpsimd.dma_start(whd_sb[:], whd_ext[:])
            nc.sync.dma_start(bhd_sb[:], bhd_ext[:].unsqueeze(-1))

            # conv moving operand: A^T chunks (bf16 + optional fp8 x64)
            AT = cpool.tile([128, NC4, N], BF16, tag="AT")
            if USE_FP8:
                at8 = cpool.tile([128, 2, 2, N], FP8, tag="at8")
            # x side: axt[64Gr + 13k + (2j+f), b, n] = (A@x_t)^T[f, n]
            # for t=6k+j; partition 13k+12 = softmax row sums (=1.0)
            axt = cpool.tile([128, NB, N], BF16, tag="axt")

            # ---- adjacency precompute ----
            with tc.tile_pool(name="pre", bufs=1) as pre:
                with tc.tile_pool(name="ppreA", bufs=2,
                                  space="PSUM") as ppre:
                    et_sb = pre.tile([ED, N], F32R, tag="et")
                    w1_sb = pre.tile([ED, ED], F32R, tag="w1")
                    w2_sb = pre.tile([ED, ED], F32R, tag="w2")
                    nc.sync.dma_start(et_sb[:], et_ext[:].bitcast(F32R))
                    nc.sync.dma_start(w1_sb[:], w1_ext[:].bitcast(F32R))
                    nc.sync.dma_start(w2_sb[:], w2_ext[:].bitcast(F32R))

                    m1t = pre.tile([ED, N], F32R, tag="m1t")
                    m2t = pre.tile([ED, N], F32R, tag="m2t")
                    for wsb, mt in ((w1_sb, m1t), (w2_sb, m2t)):
                        ps = ppre.tile([ED, N], F32, tag="mps")
                        nc.tensor.matmul(ps[:], wsb[:], et_sb[:], start=True,
                                         stop=True)
                        nc.scalar.copy(mt[:], ps[:])

                    A_sb = [pre.tile([128, N], F32, tag=f"A{i}", name=f"A{i}")
                            for i in range(NC4)]
                    for i in range(NC4):
                        ps = ppre.tile([128, N], F32, tag="sps")
                        nc.tensor.matmul(ps[:], m1t[:, i * 128:(i + 1) * 128],
                                         m2t[:], start=True, stop=True)
                        s_sb = pre.tile([128, N], F32, tag="s")
                        nc.scalar.activation(s_sb[:], ps[:], AF.Relu)
                        mx = pre.tile([128, 1], F32, tag="mx")
                        nc.vector.tensor_reduce(mx[:], s_sb[:],
                                                axis=mybir.AxisListType.X,
                                                op=ALU.max)
                        nmx = pre.tile([128, 1], F32, tag="nmx")
                        nc.vector.tensor_scalar_mul(nmx[:], mx[:], -1.0)
                        sm = pre.tile([128, 1], F32, tag="sm")
                        nc.scalar.activation(A_sb[i][:], s_sb[:], AF.Exp,
                                             bias=nmx[:], accum_out=sm[:])
                        rs = pre.tile([128, 1], F32, tag="rs")
                        nc.vector.reciprocal(rs[:], sm[:])
                        nc.vector.tensor_scalar_mul(A_sb[i][:], A_sb[i][:],
                                                    rs[:])

                    # AT chunks via 16 PE transposes
                    for c in range(NC4):
                        for i in range(NC4):
                            tp = ppre.tile([128, 128], F32, tag="tp")
                            nc.tensor.transpose(
                                tp[:], A_sb[i][:, c * 128:(c + 1) * 128],
                                ident[:])
                            nc.vector.tensor_copy(
                                AT[:, c, i * 128:(i + 1) * 128], tp[:])
                            if USE_FP8:
                                nc.scalar.mul(at8[:, c // 2, c % 2,
                                                  i * 128:(i + 1) * 128],
                                              tp[:], ASCALE)

                # ---- AXT precompute: (A@x_t)^T rows ----
                with tc.tile_pool(name="ppreB", bufs=2,
                                  space="PSUM") as ppreB:
                    xn_sb = [pre.tile([128, NB, NK, 13], BF16, tag=f"xn{c}",
                                      name=f"xn{c}") for c in range(NC4)]
                    for c in range(NC4):
                        nc.gpsimd.dma_start(xn_sb[c][:], xn_ext[c])
                    for b in range(NB):
                        axps = ppreB.tile([KX, N], F32, tag="axps",
                                          name=f"axps{b}")
                        for c in range(NC4):
                            nc.tensor.matmul(axps[:], xn_sb[c][:, b, :, :],
                                             AT[:, c, :], start=(c == 0),
                                             stop=(c == NC4 - 1))
                        nc.vector.tensor_copy(axt[0:KX, b, :], axps[:])

            # ---- scan over L ----
            _st = _ExitStack()
            spool = _st.enter_context(tc.tile_pool(name="state", bufs=2))
            wpool = _st.enter_context(tc.tile_pool(name="work", bufs=3))
            pconv = _st.enter_context(
                tc.tile_pool(name="pconv", bufs=2, space="PSUM"))
            pzr = _st.enter_context(
                tc.tile_pool(name="pzr", bufs=2, space="PSUM"))
            pht = _st.enter_context(
                tc.tile_pool(name="pht", bufs=2, space="PSUM"))

            hbf_prev = None
            h8_prev = None
            CONV8 = USE_FP8
            DR = mybir.MatmulPerfMode.DoubleRow
            FA = ACT_COPY_COLS

            def evac(dst, src_ps, tname):
                """psum -> sbuf copy, columns split between ACT and DVE."""
                nc.scalar.copy(dst[:, 0:FA], src_ps[:, 0:FA])
                nc.vector.tensor_copy(dst[:, FA:], src_ps[:, FA:])

            for t in range(L):
                hbf_new = spool.tile([128, NB, NC4, H], BF16, tag="hbf",
                                     name=f"hbf{t}")
                if CONV8:
                    h8_new = spool.tile([128, NB, NC4, H], FP8, tag="h8",
                                        name=f"h8{t}")
                for p in range(NB // 2):
                    b0 = 2 * p
                    zrsb = wpool.tile([128, 2, NC4, 2 * H], BF16, tag="zrsb",
                                      name=f"zrsb{t}_{p}")
                    htsb = wpool.tile([128, 2, NC4, H], BF16, tag="htsb",
                                      name=f"htsb{t}_{p}")
                    gsb = {}
                    for bi in range(2):
                        b = b0 + bi
                        zrps = pzr.tile([128, NC4, 2 * H], F32, tag="zr",
                                        name=f"zrps{t}_{b}")
                        if t > 0:
                            # conv1: G^T = (A @ h)^T accumulated in psum
                            gps = pconv.tile([H, N], F32, tag="cv",
                                             name=f"gps{t}_{b}")
                            if CONV8:
                                for p2 in range(2):
                                    nc.tensor.matmul(
                                        gps[:],
                                        h8_prev[:, b, 2 * p2:2 * p2 + 2, :],
                                        at8[:, p2, :, :],
                                        start=(p2 == 0), stop=(p2 == 1),
                                        perf_mode=DR)
                            else:
                                for c in range(NC4):
                                    nc.tensor.matmul(
                                        gps[:], hbf_prev[:, b, c, :],
                                        AT[:, c, :],
                                        start=(c == 0), stop=(c == NC4 - 1))
                            g_sb = wpool.tile([H, N], BF16, tag="gsb",
                                              name=f"gsb{t}_{b}")
                            evac(g_sb, gps, f"g{t}_{b}")
                            gsb[bi] = g_sb
                        # z|r gates: per bank, x(+bias) then main
                        for ca, cb in ((0, 2), (1, 3)):
                            for c in (ca, cb):
                                nc.tensor.matmul(
                                    zrps[:, c, :],
                                    axt[0:KX, b, c * 128:(c + 1) * 128],
                                    wxzr_sb[0:KX, t, :],
                                    start=True, stop=(t == 0))
                            if t > 0:
                                for c in (ca, cb):
                                    nc.tensor.matmul(
                                        zrps[:, c, :],
                                        gsb[bi][:, c * 128:(c + 1) * 128],
                                        wzr_sb[:], start=False, stop=True)
                        nc.scalar.activation(
                            zrsb[:, bi, :, :].rearrange("p a b -> p (a b)"),
                            zrps[:].rearrange("p a b -> p (a b)"), AF.Sigmoid)
                    # r*h -> conv2 lhsT (DVE)
                    if t > 0:
                        rh = spool.tile([128, 2, NC4, H],
                                        FP8 if CONV8 else BF16, tag="rh",
                                        name=f"rh{t}_{p}")
                        nc.vector.tensor_mul(
                            rh[:], zrsb[:, :, :, H:2 * H],
                            hbf_prev[:, b0:b0 + 2, :, :])
                    for bi in range(2):
                        b = b0 + bi
                        htps = pht.tile([128, NC4, H], F32, tag="ht",
                                        name=f"htps{t}_{b}")
                        if t > 0:
                            ghps = pconv.tile([H, N], F32, tag="cv",
                                              name=f"ghps{t}_{b}")
                            if CONV8:
                                for p2 in range(2):
                                    nc.tensor.matmul(
                                        ghps[:],
                                        rh[:, bi, 2 * p2:2 * p2 + 2, :],
                                        at8[:, p2, :, :],
                                        start=(p2 == 0), stop=(p2 == 1),
                                        perf_mode=DR)
                            else:
                                for c in range(NC4):
                                    nc.tensor.matmul(
                                        ghps[:], rh[:, bi, c, :], AT[:, c, :],
                                        start=(c == 0), stop=(c == NC4 - 1))
                            gh_sb = wpool.tile([H, N], BF16, tag="ghsb",
                                               name=f"ghsb{t}_{b}")
                            evac(gh_sb, ghps, f"gh{t}_{b}")
                        # candidate gate: x(+bias) then main, bank-serial
                        for c in range(NC4):
                            nc.tensor.matmul(
                                htps[:, c, :],
                                axt[0:KX, b, c * 128:(c + 1) * 128],
                                wxh_sb[0:KX, t, :],
                                start=True, stop=(t == 0))
                            if t > 0:
                                nc.tensor.matmul(
                                    htps[:, c, :],
                                    gh_sb[:, c * 128:(c + 1) * 128],
                                    whh_sb[:], start=False, stop=True)
                        nc.scalar.activation(
                            htsb[:, bi, :, :].rearrange("p a b -> p (a b)"),
                            htps[:].rearrange("p a b -> p (a b)"), AF.Tanh)
                    # GRU update: hnew = h + z*(ht - h)
                    z_ap = zrsb[:, :, :, 0:H]
                    hn_ap = hbf_new[:, b0:b0 + 2, :, :]
                    if t == 0:
                        nc.vector.tensor_mul(hn_ap, z_ap, htsb[:])
                    else:
                        d = wpool.tile([128, 2, NC4, H], BF16, tag="d",
                                       name=f"d{t}_{p}")
                        nc.vector.tensor_sub(d[:], htsb[:],
                                             hbf_prev[:, b0:b0 + 2, :, :])
                        nc.vector.tensor_mul(d[:], z_ap, d[:])
                        nc.vector.tensor_add(hn_ap,
                                             hbf_prev[:, b0:b0 + 2, :, :],
                                             d[:])
                    if CONV8:
                        nc.vector.tensor_copy(h8_new[:, b0:b0 + 2, :, :],
                                              hn_ap)
                hbf_prev = hbf_new
                if CONV8:
                    h8_prev = h8_new

            _st.close()

            # ---- head: out = (h @ Whead + bhead)^T ----
            with tc.tile_pool(name="hd", bufs=2) as hp, \
                 tc.tile_pool(name="phd", bufs=2, space="PSUM") as ph:
                for b in range(NB):
                    tp = ph.tile([128, N], BF16, tag="tp", name=f"tp{b}")
                    for c in range(NC4):
                        nc.tensor.transpose(tp[:, c * 128:(c + 1) * 128],
                                            hbf_prev[:, b, c, :], ident_b[:])
                    hT = hp.tile([128, N], BF16, tag="hT", name=f"hT{b}")
                    nc.vector.tensor_copy(hT[:], tp[:])
                    hd = ph.tile([HOR, N], F32, tag="hd", name=f"hd{b}")
                    nc.tensor.matmul(hd[:], whd_sb[:], hT[:], start=True,
                                     stop=True)
                    o_sb = hp.tile([HOR, N], F32, tag="o", name=f"o{b}")
                    nc.scalar.activation(o_sb[:], hd[:], AF.Identity,
                                         bias=bhd_sb[:])
                    nc.sync.dma_start(out_ext[b], o_sb[:])

    nc.compile()
    return nc


_NC_CACHE = None


def _get_nc():
    global _NC_CACHE
    if _NC_CACHE is None:
        _NC_CACHE = build_nc()
    return _NC_CACHE


def make_in_maps(X, E, W1, W2, Wz, bz, Wr, br, Wh, bh, Whead, bhead):
    f32 = np.float32
    Wz, Wr, Wh = np.asarray(Wz, f32), np.asarray(Wr, f32), np.asarray(Wh, f32)
    bz, br, bh = np.asarray(bz, f32), np.asarray(br, f32), np.asarray(bh, f32)
    scale = 1.0 / ASCALE if USE_FP8 else 1.0
    wzr = np.concatenate([Wz[F:], Wr[F:]], axis=1) * scale       # [H, 2H]
    whh = Wh[F:] * scale                                          # [H, H]
    # per-t zero-padded x-side stationaries over the K=52 axt rows
    wxzr = np.zeros((128, L, 2 * H), f32)
    wxh = np.zeros((128, L, H), f32)
    xzr = np.concatenate([Wz[0:F], Wr[0:F]], axis=1)              # [2, 2H]
    bzr = np.concatenate([bz, br])                                # [2H]
    for gr in range(2):
        for t in range(L):
            k, j = t // NJ, t % NJ
            r0 = 64 * gr + 13 * k
            wxzr[r0 + 2 * j:r0 + 2 * j + 2, t, :] = xzr
            wxzr[r0 + 12, t, :] = bzr
            wxh[r0 + 2 * j:r0 + 2 * j + 2, t, :] = Wh[0:F]
            wxh[r0 + 12, t, :] = bh
    ET = np.ascontiguousarray(np.asarray(E, f32).T)
    common = {
        "ET": ET, "W1": np.asarray(W1, f32), "W2": np.asarray(W2, f32),
        "WZR": np.ascontiguousarray(wzr), "WHH": np.ascontiguousarray(whh),
        "WXZR": wxzr, "WXH": wxh,
        "Whead": np.asarray(Whead, f32), "bhead": np.asarray(bhead, f32),
    }
    X = np.asarray(X, f32)
    in_maps = []
    for i in range(NCORES):
        xs = X[i * NB:(i + 1) * NB]                     # [NB, L, N, F]
        # xn[c, ki, b, k, 2j+f] = xs[b, 6k+j, 128c+ki, f]; col 12 = 1.0
        xr = xs.transpose(2, 0, 1, 3).reshape(NC4, 128, NB, NK, NJ * F)
        xn = np.zeros((NC4, 128, NB, NK, 13), f32)
        xn[..., :NJ * F] = xr
        xn[..., 12] = 1.0
        in_maps.append({"Xn": xn, **common})
    return in_maps


def run_on_hw(inputs, **kwargs):
    nc = _get_nc()
    in_maps = make_in_maps(**inputs)
    last_err = None
    for _attempt in range(3):
        try:
            res = run_bass_kernel_spmd(nc, in_maps,
                                       core_ids=list(range(NCORES)), **kwargs)
            break
        except Exception as e:  # transient NRT_EXEC_UNIT_UNRECOVERABLE
            last_err = e
            if "UNRECOVERABLE" not in str(e) and "UNAVAILABLE" not in str(e):
                raise
    else:
        raise last_err
    out = np.concatenate([res.results[i]["out"] for i in range(NCORES)], axis=0)
    return out, res


def kernel(**inputs) -> np.ndarray:
    out, _ = run_on_hw(inputs)
    return out
